# revision 12
# baseline (speedup 1.0000x reference)
"""Trainium2 Bass kernel for the CoincidenceDetector problem.

Math (reference):
    pt = 20 - 15*sigmoid(patterns)                     # (N, D)
    dt = qt[b,d] - pt[n,d]
    kappa = where(|dt| < 5, exp(-|dt|/3), 0)
    S[b,n] = sum_d kappa * |w[d]|

Device formulation (per core):
    s  = sigmoid(patterns)            (fp16, d on partitions, n on free dim)
    q'' = (20 - qt)/15                (host-precomputed, f32 per-partition scalar)
    a' = |s - q''|                    => |dt| = 15*a', window is a' < 1/3
    E  = exp(-5*a')                   (ScalarE activation, scale=-5)
    K' = relu(E - t),  M = (a' < 1/3)             with t = exp(-5/3)
    kappa = K' + t*M   (exact identity for the windowed exponential)
    S[b,:] = sum_d w*K' + sum_d (t*w)*M           (TensorE matmuls with one-hot
                                                   stationaries accumulating all
                                                   b rows in one PSUM region)

Sharding over 8 cores: patterns' N split 4 ways, batch split 2 ways.
Each core computes a (32, 4096) block of the (64, 16384) output.
"""

import numpy as np

import concourse.bass as bass
import concourse.mybir as mybir
import concourse.tile as tile
from concourse.bass_utils import run_bass_kernel_spmd

F32 = mybir.dt.float32
F16 = mybir.dt.float16
AF = mybir.ActivationFunctionType
ALU = mybir.AluOpType

B, N, D = 64, 16384, 256
N_SPLIT, B_SPLIT = 4, 2
N_CORES = N_SPLIT * B_SPLIT
P = 128
DBLK = D // P  # 2
CHUNK = 512    # one PSUM bank of fp32

T_CUT = float(np.float32(np.exp(np.float32(-5.0) / np.float32(3.0))))
ONE_THIRD = float(np.float32(1.0) / np.float32(3.0))

_PROGRAM_CACHE = {}


def _split_multi_waits(nc, max_inline=1):
    """The walrus codegen in this container supports only a small number of
    embedded sync-wait commands per instruction (1 for most engine ops).
    Tile's wait assignment can attach several.  Hoist all but `max_inline`
    waits of every instruction onto standalone EventSemaphore instructions
    (one wait each) inserted immediately before it on the same engine —
    semantically identical, the engine queue stalls the same way."""
    for bbname, bass_bb in list(nc.bb_map.items()):
        insts = bass_bb.bb.instructions
        i = 0
        while i < len(insts):
            inst = insts[i]
            si = inst.sync_info
            if si is not None and si.on_wait and len(si.on_wait) > max_inline:
                waits = list(si.on_wait)
                keep = waits[-max_inline:] if max_inline else []
                hoist = waits[: len(waits) - max_inline]
                carriers = []
                for w in hoist:
                    ev = mybir.InstEventSemaphore(
                        name=nc.get_next_instruction_name(),
                        engine=inst.engine,
                        ins=[],
                        outs=[],
                        sync_info=mybir.SyncInfo(on_wait=[w], on_update=[]),
                    )
                    nc.register_instruction(ev)
                    carriers.append(ev)
                inst.sync_info = mybir.SyncInfo(
                    on_wait=keep, on_update=list(si.on_update)
                )
                insts[i:i] = carriers
                i += len(carriers)
            i += 1


def build_program(n_loc=N // N_SPLIT, b_loc=B // B_SPLIT, pre_chunk=1024,
                  act_abs_every=5, repeat=1):
    """Build the single-core Bass program (same program runs SPMD on all cores).

    repeat > 1 re-runs the whole compute (sigmoid + main loop + PSUM drain)
    that many times — used only for wall-clock differential timing."""
    nch = n_loc // CHUNK
    nc = bass.Bass("TRN2")

    ptT = nc.dram_tensor("ptt", [DBLK, P, n_loc], F32, kind="ExternalInput")
    qtT = nc.dram_tensor("qtt", [DBLK, P, b_loc], F32, kind="ExternalInput")
    wko = nc.dram_tensor("wko", [P, DBLK * b_loc * b_loc], F16, kind="ExternalInput")
    wmo = nc.dram_tensor("wmo", [P, DBLK * b_loc * b_loc], F16, kind="ExternalInput")
    out = nc.dram_tensor("out", [b_loc, n_loc], F32, kind="ExternalOutput")

    with tile.TileContext(nc) as tc:
        with (
            tc.tile_pool(name="const", bufs=1) as constp,
            tc.tile_pool(name="pre", bufs=1) as prep,
            tc.tile_pool(name="sig", bufs=1) as sigp,
            tc.tile_pool(name="work", bufs=3) as wp,
            tc.tile_pool(name="outp", bufs=1) as outp,
            tc.tile_pool(name="psum", bufs=1, space="PSUM") as pp,
        ):
            # --- constants ---
            qt_sb = constp.tile([P, DBLK * b_loc], F32, tag="qt")
            for d in range(DBLK):
                nc.sync.dma_start(qt_sb[:, d * b_loc:(d + 1) * b_loc], qtT[d])
            wk_sb = constp.tile([P, DBLK * b_loc * b_loc], F16, tag="wk")
            nc.sync.dma_start(wk_sb[:], wko[:])
            wm_sb = constp.tile([P, DBLK * b_loc * b_loc], F16, tag="wm")
            nc.sync.dma_start(wm_sb[:], wmo[:])

            # --- preprocessing: s = sigmoid(patterns_T), fp16, chunked DMA overlap ---
            s_sb = [
                sigp.tile([P, n_loc], F16, tag=f"s{d}", name=f"s{d}")
                for d in range(DBLK)
            ]
            ptf_sb = [
                prep.tile([P, n_loc], F32, tag=f"ptf{d}", name=f"ptf{d}")
                for d in range(DBLK)
            ]
            for d in range(DBLK):
                for j in range(0, n_loc, pre_chunk):
                    nc.sync.dma_start(
                        ptf_sb[d][:, j:j + pre_chunk], ptT[d][:, j:j + pre_chunk]
                    )

            psum_acc = pp.tile([b_loc, n_loc], F32, tag="acc")
            s_out = outp.tile([b_loc, n_loc], F32, tag="so")
            n_iter = DBLK * b_loc

            for _rep in range(repeat):
                for d in range(DBLK):
                    for j in range(0, n_loc, pre_chunk):
                        nc.scalar.activation(
                            s_sb[d][:, j:j + pre_chunk],
                            ptf_sb[d][:, j:j + pre_chunk],
                            AF.Sigmoid,
                        )

                # --- main loop ---
                it = 0
                for d in range(DBLK):
                    for b in range(b_loc):
                        col = d * b_loc + b
                        a_t = wp.tile([P, n_loc], F16, tag="a")
                        if it % act_abs_every == act_abs_every - 1:
                            # ScalarE path: a' = |(-1)*s + q''| in one ACT pass
                            # (rebalances DVE vs ScalarE load)
                            nc.scalar.activation(
                                a_t[:], s_sb[d][:], AF.Abs,
                                bias=qt_sb[:, col:col + 1], scale=-1.0,
                            )
                        else:
                            # DVE path: d1 = s - q'' ; a' = d1 & 0x7fff (abs)
                            d_t = wp.tile([P, n_loc], F16, tag="d")
                            nc.vector.tensor_scalar(
                                d_t[:], s_sb[d][:],
                                qt_sb[:, col:col + 1], None,
                                ALU.subtract,
                            )
                            nc.vector.tensor_scalar(
                                a_t.bitcast(mybir.dt.uint16)[:],
                                d_t.bitcast(mybir.dt.uint16)[:],
                                0x7FFF, None,
                                ALU.bitwise_and,
                            )
                        e_t = wp.tile([P, n_loc], F16, tag="e")
                        nc.scalar.activation(e_t[:], a_t[:], AF.Exp, scale=-5.0)
                        k_t = wp.tile([P, n_loc], F16, tag="k")
                        nc.vector.tensor_scalar(
                            k_t[:], e_t[:], T_CUT, 0.0, ALU.subtract, ALU.max
                        )
                        m_t = wp.tile([P, n_loc], F16, tag="m")
                        nc.vector.tensor_scalar(
                            m_t[:], a_t[:], ONE_THIRD, None, ALU.is_lt
                        )
                        first = it == 0
                        last = it == n_iter - 1
                        for ci in range(nch):
                            sl = slice(ci * CHUNK, (ci + 1) * CHUNK)
                            nc.tensor.matmul(
                                psum_acc[:, sl],
                                wk_sb[:, col * b_loc:(col + 1) * b_loc],
                                k_t[:, sl],
                                start=first, stop=False,
                            )
                            nc.tensor.matmul(
                                psum_acc[:, sl],
                                wm_sb[:, col * b_loc:(col + 1) * b_loc],
                                m_t[:, sl],
                                start=False, stop=last,
                            )
                        it += 1

                # --- drain PSUM -> SBUF ---
                for ci in range(nch):
                    sl = slice(ci * CHUNK, (ci + 1) * CHUNK)
                    nc.vector.tensor_copy(s_out[:, sl], psum_acc[:, sl])

            nc.sync.dma_start(out[:], s_out[:])

    _split_multi_waits(nc)
    return nc


def _get_program(repeat=1):
    key = ("default", repeat)
    if key not in _PROGRAM_CACHE:
        _PROGRAM_CACHE[key] = build_program(repeat=repeat)
    return _PROGRAM_CACHE[key]


def make_in_maps(query_times, patterns, weights, n_loc, b_loc):
    """Host-side input marshalling: layout transforms only (plus the tiny
    O(B*D + D) affine/abs precomputation)."""
    qt = np.asarray(query_times, dtype=np.float32)
    pat = np.asarray(patterns, dtype=np.float32)
    w = np.asarray(weights, dtype=np.float32)

    q2T = np.ascontiguousarray(((np.float32(20.0) - qt) / np.float32(15.0)).T)  # (D, B)
    ptT = np.ascontiguousarray(pat.T)  # (D, N)
    w_abs = np.abs(w)

    # one-hot stationaries: wk[d, part, b, m] = |w|[d*128+part] * (m == b)
    wk = np.zeros((DBLK, P, b_loc, b_loc), np.float16)
    for d in range(DBLK):
        col = w_abs[d * P:(d + 1) * P].astype(np.float16)
        for b in range(b_loc):
            wk[d, :, b, b] = col
    t16 = np.float16(np.float32(T_CUT))
    wm = (wk.astype(np.float32) * np.float32(T_CUT)).astype(np.float16)
    wk_flat = np.ascontiguousarray(wk.transpose(1, 0, 2, 3).reshape(P, DBLK * b_loc * b_loc))
    wm_flat = np.ascontiguousarray(wm.transpose(1, 0, 2, 3).reshape(P, DBLK * b_loc * b_loc))

    in_maps = []
    for c in range(N_CORES):
        b_grp, n_grp = divmod(c, N_SPLIT)
        in_maps.append({
            "ptt": np.ascontiguousarray(
                ptT[:, n_grp * n_loc:(n_grp + 1) * n_loc]
            ).reshape(DBLK, P, n_loc),
            "qtt": np.ascontiguousarray(
                q2T[:, b_grp * b_loc:(b_grp + 1) * b_loc]
            ).reshape(DBLK, P, b_loc),
            "wko": wk_flat,
            "wmo": wm_flat,
        })
    return in_maps


def kernel(query_times, patterns, weights, _trace=False, _repeat=1):
    n_loc = N // N_SPLIT
    b_loc = B // B_SPLIT
    nc = _get_program(repeat=_repeat)
    in_maps = make_in_maps(query_times, patterns, weights, n_loc, b_loc)

    res = run_bass_kernel_spmd(nc, in_maps, list(range(N_CORES)), trace=_trace)

    S = np.empty((B, N), np.float32)
    for c in range(N_CORES):
        b_grp, n_grp = divmod(c, N_SPLIT)
        S[b_grp * b_loc:(b_grp + 1) * b_loc,
          n_grp * n_loc:(n_grp + 1) * n_loc] = res.results[c]["out"]
    if _trace:
        return S, res
    return S


# revision 13
# speedup vs baseline: 4.1397x; 4.1397x over previous
"""Trainium2 Bass kernel for the CoincidenceDetector problem.

Math (reference):
    pt = 20 - 15*sigmoid(patterns)                     # (N, D)
    dt = qt[b,d] - pt[n,d]
    kappa = where(|dt| < 5, exp(-|dt|/3), 0)
    S[b,n] = sum_d kappa * |w[d]|

Device formulation (per core, patterns' N sharded 8 ways, n on partitions):
    s  = sigmoid(patterns)                   (fp16; |dt| = 15*|s - q''|)
    q'' = (20 - qt)/15                       (host-precomputed, replicated
                                              across partitions, fp16)
    per 128-pattern tile (free dim = 64 batches x 256 features = 16384):
      d1    = s (broadcast over b) - q''               DVE tensor_tensor
      a'    = d1 & 0x7fff   (abs via sign-bit clear)   DVE tensor_scalar
      E     = exp(-5*a')                               ScalarE activation
      M     = a' < 1/3      (coincidence window)       DVE tensor_scalar
      kappa = E * M                                    DVE tensor_tensor
      S     = reduce_add(kappa, over d)                DVE tensor_reduce
    -> (128 patterns, 64 batches) fp32 scores per tile.

The execution environment prices instructions (~30-60us each) far above
their architectural cost, so the kernel is shaped to minimize instruction
count: ~100 instructions/core instead of a matmul-based formulation.
"""

import numpy as np

import concourse.bass as bass
import concourse.mybir as mybir
import concourse.tile as tile
from concourse.bass_utils import run_bass_kernel_spmd

F32 = mybir.dt.float32
F16 = mybir.dt.float16
U16 = mybir.dt.uint16
AF = mybir.ActivationFunctionType
ALU = mybir.AluOpType

B, N, D = 64, 16384, 256
N_SPLIT = 8
N_CORES = 8
P = 128

ONE_THIRD = float(np.float32(1.0) / np.float32(3.0))

_PROGRAM_CACHE = {}


def _split_multi_waits(nc, max_inline=1):
    """The walrus codegen in this container supports only a small number of
    embedded sync-wait commands per instruction (1 for most engine ops).
    Tile's wait assignment can attach several.  Hoist all but `max_inline`
    waits of every instruction onto standalone EventSemaphore instructions
    (one wait each) inserted immediately before it on the same engine —
    semantically identical, the engine queue stalls the same way."""
    for bbname, bass_bb in list(nc.bb_map.items()):
        insts = bass_bb.bb.instructions
        i = 0
        while i < len(insts):
            inst = insts[i]
            si = inst.sync_info
            if si is not None and si.on_wait and len(si.on_wait) > max_inline:
                waits = list(si.on_wait)
                keep = waits[-max_inline:] if max_inline else []
                hoist = waits[: len(waits) - max_inline]
                carriers = []
                for w in hoist:
                    ev = mybir.InstEventSemaphore(
                        name=nc.get_next_instruction_name(),
                        engine=inst.engine,
                        ins=[],
                        outs=[],
                        sync_info=mybir.SyncInfo(on_wait=[w], on_update=[]),
                    )
                    nc.register_instruction(ev)
                    carriers.append(ev)
                inst.sync_info = mybir.SyncInfo(
                    on_wait=keep, on_update=list(si.on_update)
                )
                insts[i:i] = carriers
                i += len(carriers)
            i += 1


def build_program(n_loc=N // N_SPLIT, b_loc=B, repeat=1, with_weights=False):
    """Build the single-core Bass program (same program runs SPMD on all
    cores; per-core data differs only in the patterns shard).

    repeat > 1 re-runs the whole compute that many times — used only for
    wall-clock differential timing."""
    ntile = n_loc // P           # 16
    fd = b_loc * D               # 16384
    nc = bass.Bass("TRN2")

    patn = nc.dram_tensor("patn", [P, ntile * D], F32, kind="ExternalInput")
    qrep = nc.dram_tensor("qrep", [P, fd], F16, kind="ExternalInput")
    wrep = None
    if with_weights:
        wrep = nc.dram_tensor("wrep", [P, D], F16, kind="ExternalInput")
    out = nc.dram_tensor("out", [P, ntile * b_loc], F32, kind="ExternalOutput")

    with tile.TileContext(nc) as tc:
        with (
            tc.tile_pool(name="const", bufs=1) as constp,
            tc.tile_pool(name="work", bufs=1) as wp,
        ):
            pat_sb = constp.tile([P, ntile * D], F32, tag="pat", name="pat")
            nc.sync.dma_start(pat_sb[:], patn[:])
            q_sb = constp.tile([P, fd], F16, tag="q", name="q")
            nc.sync.dma_start(q_sb[:], qrep[:])
            w_sb = None
            if with_weights:
                w_sb = constp.tile([P, D], F16, tag="w", name="w")
                nc.sync.dma_start(w_sb[:], wrep[:])
            s_sb = constp.tile([P, ntile * D], F16, tag="s", name="s")
            s_out = constp.tile([P, ntile * b_loc], F32, tag="so", name="so")

            q_3d = q_sb[:].rearrange("p (b d) -> p b d", b=b_loc)

            for _rep in range(repeat):
                nc.scalar.activation(s_sb[:], pat_sb[:], AF.Sigmoid)
                for t in range(ntile):
                    s_bc = (
                        s_sb[:, t * D:(t + 1) * D]
                        .rearrange("p (u d) -> p u d", u=1)
                        .broadcast_to([P, b_loc, D])
                    )
                    d1 = wp.tile([P, fd], F16, tag="d1", name="d1")
                    d1_3d = d1[:].rearrange("p (b d) -> p b d", b=b_loc)
                    nc.vector.tensor_tensor(
                        out=d1_3d, in0=s_bc, in1=q_3d, op=ALU.subtract
                    )
                    # abs in place: clear the fp16 sign bit
                    nc.vector.tensor_scalar(
                        d1.bitcast(U16)[:], d1.bitcast(U16)[:],
                        0x7FFF, None, ALU.bitwise_and,
                    )
                    e_t = wp.tile([P, fd], F16, tag="e", name="e")
                    nc.scalar.activation(e_t[:], d1[:], AF.Exp, scale=-5.0)
                    m_t = wp.tile([P, fd], F16, tag="m", name="m")
                    nc.vector.tensor_scalar(
                        m_t[:], d1[:], ONE_THIRD, None, ALU.is_lt
                    )
                    # kappa = E * M, in place over M
                    m_3d = m_t[:].rearrange("p (b d) -> p b d", b=b_loc)
                    nc.vector.tensor_tensor(
                        out=m_3d,
                        in0=e_t[:].rearrange("p (b d) -> p b d", b=b_loc),
                        in1=m_3d, op=ALU.mult,
                    )
                    if with_weights:
                        w_bc = (
                            w_sb[:]
                            .rearrange("p (u d) -> p u d", u=1)
                            .broadcast_to([P, b_loc, D])
                        )
                        nc.vector.tensor_tensor(
                            out=m_3d, in0=m_3d, in1=w_bc, op=ALU.mult
                        )
                    nc.vector.tensor_reduce(
                        out=s_out[:, t * b_loc:(t + 1) * b_loc],
                        in_=m_3d,
                        axis=mybir.AxisListType.X,
                        op=ALU.add,
                    )

            nc.sync.dma_start(out[:], s_out[:])

    _split_multi_waits(nc)
    return nc


def _get_program(repeat=1, with_weights=False):
    key = (repeat, with_weights)
    if key not in _PROGRAM_CACHE:
        _PROGRAM_CACHE[key] = build_program(
            repeat=repeat, with_weights=with_weights
        )
    return _PROGRAM_CACHE[key]


def make_in_maps(query_times, patterns, weights, n_loc, b_loc,
                 with_weights=False):
    """Host-side input marshalling: layout transforms plus the tiny
    O(B*D + D) affine precomputation."""
    qt = np.asarray(query_times, dtype=np.float32)
    pat = np.asarray(patterns, dtype=np.float32)
    w = np.asarray(weights, dtype=np.float32)
    ntile = n_loc // P

    q2 = ((np.float32(20.0) - qt) / np.float32(15.0)).astype(np.float16)
    qrep = np.ascontiguousarray(
        np.broadcast_to(q2.reshape(1, b_loc * D), (P, b_loc * D))
    )
    shared = {"qrep": qrep}
    if with_weights:
        w16 = np.abs(w).astype(np.float16)
        shared["wrep"] = np.ascontiguousarray(
            np.broadcast_to(w16.reshape(1, D), (P, D))
        )

    in_maps = []
    for c in range(N_CORES):
        shard = pat[c * n_loc:(c + 1) * n_loc]  # (n_loc, D)
        patn = np.ascontiguousarray(
            shard.reshape(ntile, P, D).transpose(1, 0, 2).reshape(P, ntile * D)
        )
        in_maps.append({"patn": patn, **shared})
    return in_maps


def kernel(query_times, patterns, weights, _trace=False, _repeat=1):
    n_loc = N // N_SPLIT
    b_loc = B
    ntile = n_loc // P

    w = np.asarray(weights, dtype=np.float32)
    with_weights = not np.all(np.abs(w) == 1.0)

    nc = _get_program(repeat=_repeat, with_weights=with_weights)
    in_maps = make_in_maps(query_times, patterns, weights, n_loc, b_loc,
                           with_weights=with_weights)

    res = run_bass_kernel_spmd(nc, in_maps, list(range(N_CORES)), trace=_trace)

    S = np.empty((B, N), np.float32)
    for c in range(N_CORES):
        o = res.results[c]["out"]  # (P, ntile*b_loc)
        o = o.reshape(P, ntile, b_loc).transpose(2, 1, 0).reshape(B, n_loc)
        S[:, c * n_loc:(c + 1) * n_loc] = o
    if _trace:
        return S, res
    return S


# revision 14
# speedup vs baseline: 12.9532x; 3.1290x over previous
"""Trainium2 Bass kernel for the CoincidenceDetector problem.

Math (reference):
    pt = 20 - 15*sigmoid(patterns)                     # (N, D)
    dt = qt[b,d] - pt[n,d]
    kappa = where(|dt| < 5, exp(-|dt|/3), 0)
    S[b,n] = sum_d kappa * |w[d]|

Device formulation (per core, patterns' N sharded 8 ways, n on partitions):
    s  = sigmoid(patterns)                   (fp16; |dt| = 15*|s - q''|)
    q'' = (20 - qt)/15                       (host-precomputed, replicated
                                              across partitions, fp16)
    per 128-pattern tile (free dim = 64 batches x 256 features = 16384):
      d1    = s (broadcast over b) - q''               DVE tensor_tensor
      a'    = d1 & 0x7fff   (abs via sign-bit clear)   DVE tensor_scalar
      E     = exp(-5*a')                               ScalarE activation
      M     = a' < 1/3      (coincidence window)       DVE tensor_scalar
      kappa = E * M                                    DVE tensor_tensor
      S     = reduce_add(kappa, over d)                DVE tensor_reduce
    -> (128 patterns, 64 batches) fp32 scores per tile.

The execution environment prices instructions (~30-60us each) far above
their architectural cost, so the kernel is shaped to minimize instruction
count: ~100 instructions/core instead of a matmul-based formulation.
"""

import numpy as np

import concourse.bass as bass
import concourse.mybir as mybir
import concourse.tile as tile
from concourse.bass_utils import run_bass_kernel_spmd

F32 = mybir.dt.float32
F16 = mybir.dt.float16
U16 = mybir.dt.uint16
AF = mybir.ActivationFunctionType
ALU = mybir.AluOpType

B, N, D = 64, 16384, 256
N_SPLIT = 8
N_CORES = 8
P = 128

ONE_THIRD = float(np.float32(1.0) / np.float32(3.0))

_PROGRAM_CACHE = {}


def _split_multi_waits(nc, max_inline=1):
    """The walrus codegen in this container supports only a small number of
    embedded sync-wait commands per instruction (1 for most engine ops).
    Tile's wait assignment can attach several.  Hoist all but `max_inline`
    waits of every instruction onto standalone EventSemaphore instructions
    (one wait each) inserted immediately before it on the same engine —
    semantically identical, the engine queue stalls the same way."""
    for bbname, bass_bb in list(nc.bb_map.items()):
        insts = bass_bb.bb.instructions
        i = 0
        while i < len(insts):
            inst = insts[i]
            si = inst.sync_info
            if si is not None and si.on_wait and len(si.on_wait) > max_inline:
                waits = list(si.on_wait)
                keep = waits[-max_inline:] if max_inline else []
                hoist = waits[: len(waits) - max_inline]
                carriers = []
                for w in hoist:
                    ev = mybir.InstEventSemaphore(
                        name=nc.get_next_instruction_name(),
                        engine=inst.engine,
                        ins=[],
                        outs=[],
                        sync_info=mybir.SyncInfo(on_wait=[w], on_update=[]),
                    )
                    nc.register_instruction(ev)
                    carriers.append(ev)
                inst.sync_info = mybir.SyncInfo(
                    on_wait=keep, on_update=list(si.on_update)
                )
                insts[i:i] = carriers
                i += len(carriers)
            i += 1


def build_program(n_loc=N // N_SPLIT, b_loc=B, repeat=1, with_weights=False):
    """Build the single-core Bass program (same program runs SPMD on all
    cores; per-core data differs only in the patterns shard).

    repeat > 1 re-runs the whole compute that many times — used only for
    wall-clock differential timing."""
    ntile = n_loc // P           # 16
    fd = b_loc * D               # 16384
    nc = bass.Bass("TRN2")

    patn = nc.dram_tensor("patn", [P, ntile * D], F32, kind="ExternalInput")
    qrep = nc.dram_tensor("qrep", [P, fd], F16, kind="ExternalInput")
    wrep = None
    if with_weights:
        wrep = nc.dram_tensor("wrep", [P, D], F16, kind="ExternalInput")
    out = nc.dram_tensor("out", [P, ntile * b_loc], F32, kind="ExternalOutput")

    with tile.TileContext(nc) as tc:
        with (
            tc.tile_pool(name="const", bufs=1) as constp,
            tc.tile_pool(name="work", bufs=1) as wp,
        ):
            pat_sb = constp.tile([P, ntile * D], F32, tag="pat", name="pat")
            nc.sync.dma_start(pat_sb[:], patn[:])
            q_sb = constp.tile([P, fd], F16, tag="q", name="q")
            nc.sync.dma_start(q_sb[:], qrep[:])
            w_sb = None
            if with_weights:
                w_sb = constp.tile([P, D], F16, tag="w", name="w")
                nc.sync.dma_start(w_sb[:], wrep[:])
            s_sb = constp.tile([P, ntile * D], F16, tag="s", name="s")
            s_out = constp.tile([P, ntile * b_loc], F32, tag="so", name="so")

            q_3d = q_sb[:].rearrange("p (b d) -> p b d", b=b_loc)

            for _rep in range(repeat):
                nc.scalar.activation(s_sb[:], pat_sb[:], AF.Sigmoid)
                for t in range(ntile):
                    s_bc = (
                        s_sb[:, t * D:(t + 1) * D]
                        .rearrange("p (u d) -> p u d", u=1)
                        .broadcast_to([P, b_loc, D])
                    )
                    d1 = wp.tile([P, fd], F16, tag="d1", name="d1")
                    d1_3d = d1[:].rearrange("p (b d) -> p b d", b=b_loc)
                    nc.vector.tensor_tensor(
                        out=d1_3d, in0=s_bc, in1=q_3d, op=ALU.subtract
                    )
                    # abs in place: clear the fp16 sign bit
                    nc.vector.tensor_scalar(
                        d1.bitcast(U16)[:], d1.bitcast(U16)[:],
                        0x7FFF, None, ALU.bitwise_and,
                    )
                    e_t = wp.tile([P, fd], F16, tag="e", name="e")
                    nc.scalar.activation(e_t[:], d1[:], AF.Exp, scale=-5.0)
                    # kappa = (a' < 1/3) * E fused in one pass
                    m_t = wp.tile([P, fd], F16, tag="m", name="m")
                    m_3d = m_t[:].rearrange("p (b d) -> p b d", b=b_loc)
                    nc.vector.scalar_tensor_tensor(
                        out=m_t[:], in0=d1[:], scalar=ONE_THIRD, in1=e_t[:],
                        op0=ALU.is_lt, op1=ALU.mult,
                    )
                    if with_weights:
                        w_bc = (
                            w_sb[:]
                            .rearrange("p (u d) -> p u d", u=1)
                            .broadcast_to([P, b_loc, D])
                        )
                        nc.vector.tensor_tensor(
                            out=m_3d, in0=m_3d, in1=w_bc, op=ALU.mult
                        )
                    nc.vector.tensor_reduce(
                        out=s_out[:, t * b_loc:(t + 1) * b_loc],
                        in_=m_3d,
                        axis=mybir.AxisListType.X,
                        op=ALU.add,
                    )

            nc.sync.dma_start(out[:], s_out[:])

    _split_multi_waits(nc)
    return nc


def _get_program(repeat=1, with_weights=False):
    key = (repeat, with_weights)
    if key not in _PROGRAM_CACHE:
        _PROGRAM_CACHE[key] = build_program(
            repeat=repeat, with_weights=with_weights
        )
    return _PROGRAM_CACHE[key]


def make_in_maps(query_times, patterns, weights, n_loc, b_loc,
                 with_weights=False):
    """Host-side input marshalling: layout transforms plus the tiny
    O(B*D + D) affine precomputation."""
    qt = np.asarray(query_times, dtype=np.float32)
    pat = np.asarray(patterns, dtype=np.float32)
    w = np.asarray(weights, dtype=np.float32)
    ntile = n_loc // P

    q2 = ((np.float32(20.0) - qt) / np.float32(15.0)).astype(np.float16)
    qrep = np.ascontiguousarray(
        np.broadcast_to(q2.reshape(1, b_loc * D), (P, b_loc * D))
    )
    shared = {"qrep": qrep}
    if with_weights:
        w16 = np.abs(w).astype(np.float16)
        shared["wrep"] = np.ascontiguousarray(
            np.broadcast_to(w16.reshape(1, D), (P, D))
        )

    in_maps = []
    for c in range(N_CORES):
        shard = pat[c * n_loc:(c + 1) * n_loc]  # (n_loc, D)
        patn = np.ascontiguousarray(
            shard.reshape(ntile, P, D).transpose(1, 0, 2).reshape(P, ntile * D)
        )
        in_maps.append({"patn": patn, **shared})
    return in_maps


def kernel(query_times, patterns, weights, _trace=False, _repeat=1):
    n_loc = N // N_SPLIT
    b_loc = B
    ntile = n_loc // P

    w = np.asarray(weights, dtype=np.float32)
    with_weights = not np.all(np.abs(w) == 1.0)

    nc = _get_program(repeat=_repeat, with_weights=with_weights)
    in_maps = make_in_maps(query_times, patterns, weights, n_loc, b_loc,
                           with_weights=with_weights)

    res = run_bass_kernel_spmd(nc, in_maps, list(range(N_CORES)), trace=_trace)

    S = np.empty((B, N), np.float32)
    for c in range(N_CORES):
        o = res.results[c]["out"]  # (P, ntile*b_loc)
        o = o.reshape(P, ntile, b_loc).transpose(2, 1, 0).reshape(B, n_loc)
        S[:, c * n_loc:(c + 1) * n_loc] = o
    if _trace:
        return S, res
    return S
